# revision 33
# baseline (speedup 1.0000x reference)
"""GAT fusion Trainium2 kernel — nn_GAT_Fusion_2147483648587.

Full inputs in, full output out. Node/dst-sharded across 8 NeuronCores
(graph parallel); per-edge source features are host-gathered between
three device launches (halo exchange); all FLOPs run on device.

  launch 1: per-core dense  h1 = x@W1 (+folded attention score vectors)
  host    : gather h1[src], as1[src], ad1[dst] per edge (pure indexing)
  launch 2: L1 edge phase — exp(lrelu(scores)) weights, one-hot-mask
            matmul aggregation in PSUM, softmax-normalize, ELU,
            h2 = h1'@W2 (+folded L2 score vectors)
  host    : gather h2[src], as2[src], ad2[dst] per edge
  launch 3: L2 edge phase + fusion MLP -> out[N]

Segment softmax computed without max-subtraction (exactly equivalent —
the per-dst scale cancels between numerator and denominator; scores are
O(10) so fp32 exp cannot overflow).
"""
import sys

sys.path.insert(0, "/opt/trn_rl_repo")

import numpy as np
import ml_dtypes

import concourse.bass as bass
from concourse import bacc
import concourse.tile as tile
from concourse import mybir
from concourse import bass_utils

BF16 = ml_dtypes.bfloat16
P = 128
NCORE = 8
N = 50000
NPC = N // NCORE            # 6250 dst nodes per core
NBLK = (NPC + P - 1) // P   # 49 dst blocks per core
SLOTS = NBLK * P            # 6272
CH = 32                     # tiles per chunk (4096 edges)
dt = mybir.dt
AX = mybir.AxisListType.X
ALU = mybir.AluOpType
ACTF = mybir.ActivationFunctionType


def _ap(tile_ap, dims):
    return bass.AP(tensor=tile_ap.tensor, offset=tile_ap.offset,
                   ap=[tile_ap.ap[0]] + dims)


# ----------------------------------------------------------------- host prep
def _fold_weights(W1, a_src1, a_dst1, W2, a_src2, a_dst2):
    Ws1 = np.stack([W1[:, h * 32:(h + 1) * 32] @ a_src1[h] for h in range(4)], 1)
    Wd1 = np.stack([W1[:, h * 32:(h + 1) * 32] @ a_dst1[h] for h in range(4)], 1)
    Wc1 = np.concatenate([W1, Ws1, Wd1], axis=1)          # [512, 136]
    Ws2 = (W2 @ a_src2[0])[:, None]
    Wd2 = (W2 @ a_dst2[0])[:, None]
    Wc2 = np.concatenate([W2, Ws2, Wd2], axis=1)          # [128, 34]
    return Wc1.astype(BF16), Wc2.astype(BF16)


def _edge_prep(edge_index):
    """Round-robin identity layout: per core, dsts are permuted by
    descending degree into slots; the edge at (partition c, tile r of
    block b) targets dst slot b*128+c, so the aggregation mask is the
    constant identity. Tile counts per block shared by all cores."""
    loops = np.arange(N, dtype=np.int64)
    src = np.concatenate([edge_index[0], loops]).astype(np.int64)
    dst = np.concatenate([edge_index[1], loops]).astype(np.int64)
    deg = np.bincount(dst, minlength=N)
    owner = dst // NPC

    orders = []
    Tb = np.ones(NBLK, np.int64)
    for p in range(NCORE):
        d = deg[p * NPC:(p + 1) * NPC]
        order = np.argsort(-d, kind="stable")        # slot -> local node
        orders.append(order)
        ds = np.concatenate([d[order], np.zeros(SLOTS - NPC, np.int64)])
        Tb = np.maximum(Tb, ds.reshape(NBLK, P).max(axis=1))

    tstart = np.zeros(NBLK + 1, np.int64)
    tstart[1:] = np.cumsum(Tb)
    TT = int(tstart[-1])

    sched = []   # (blk, tile_start, ntiles, first_of_blk, last_of_blk)
    for b in range(NBLK):
        t0, rem, off = int(tstart[b]), int(Tb[b]), 0
        while rem > 0:
            nt = min(CH, rem)
            sched.append((b, t0 + off, nt, off == 0, rem - nt == 0))
            off += nt
            rem -= nt

    tile2blk = np.zeros(TT, np.int64)
    for b in range(NBLK):
        tile2blk[tstart[b]:tstart[b + 1]] = b

    pos_src, pos_pad = [], []
    for p in range(NCORE):
        rank = np.empty(NPC, np.int64)
        rank[orders[p]] = np.arange(NPC)
        sel = owner == p
        s_p = src[sel]
        slot = rank[dst[sel] - p * NPC]
        o = np.argsort(slot, kind="stable")
        s_p, slot = s_p[o], slot[o]
        run0 = np.concatenate(([0], np.cumsum(np.bincount(slot,
                                                          minlength=SLOTS))))
        r = np.arange(len(slot)) - run0[slot]        # round within slot
        tcol = tstart[slot >> 7] + r
        crow = slot & 127
        srcp = np.zeros((P, TT), np.int64)
        pad = np.ones((P, TT), bool)
        srcp[crow, tcol] = s_p
        pad[crow, tcol] = False
        pos_src.append(srcp)
        pos_pad.append(pad)
    return pos_src, pos_pad, sched, TT, tile2blk, orders


# ------------------------------------------------------------- bass builders
def _mk_nc():
    return bacc.Bacc("TRN2", target_bir_lowering=False, debug=False,
                     num_devices=NCORE)


def build_l1():
    nc = _mk_nc()
    xT = nc.dram_tensor("xT", [512, SLOTS], dt.bfloat16, kind="ExternalInput").ap()
    wc1 = nc.dram_tensor("wc1", [512, 136], dt.bfloat16, kind="ExternalInput").ap()
    h1b = nc.dram_tensor("h1b", [P, NBLK * P], dt.bfloat16,
                         kind="ExternalOutput").ap()
    sc = nc.dram_tensor("sc", [P, NBLK * 8], dt.float32,
                        kind="ExternalOutput").ap()
    with tile.TileContext(nc) as tc:
        with (
            tc.tile_pool(name="w", bufs=1) as wp,
            tc.tile_pool(name="x", bufs=4) as xp,
            tc.tile_pool(name="s", bufs=3) as sp,
            tc.tile_pool(name="ps", bufs=3, space="PSUM") as pp,
        ):
            wk = []
            for k in range(4):
                w = wp.tile([P, 136], dt.bfloat16, tag=f"w{k}")
                nc.sync.dma_start(w[:], wc1[k * P:(k + 1) * P, :])
                wk.append(w)
            GRP = 8                       # blocks per x-DMA batch
            for j0 in range(0, NBLK, GRP):
                nb = min(GRP, NBLK - j0)
                xt = xp.tile([P, 4, GRP * P], dt.bfloat16, tag="xt")
                for k in range(4):
                    nc.sync.dma_start(
                        xt[:, k, :nb * P],
                        xT[k * P:(k + 1) * P, j0 * P:(j0 + nb) * P])
                hb = sp.tile([P, GRP, P], dt.bfloat16, tag="hb")
                scb = sp.tile([P, GRP, 8], dt.float32, tag="scb")
                for j in range(j0, j0 + nb):
                    o = j - j0
                    ps = pp.tile([P, 136], dt.float32, tag="p1")
                    for k in range(4):
                        nc.tensor.matmul(out=ps[:], lhsT=xt[:, k, o * P:(o + 1) * P],
                                         rhs=wk[k][:],
                                         start=(k == 0), stop=(k == 3))
                    nc.vector.tensor_copy(hb[:, o, :], ps[:, 0:P])
                    nc.vector.tensor_copy(scb[:, o, :], ps[:, P:136])
                h1v = h1b.rearrange("p (j f) -> p j f", f=P)
                scv = sc.rearrange("p (j f) -> p j f", f=8)
                nc.sync.dma_start(h1v[:, j0:j0 + nb, :], hb[:, :nb, :])
                nc.sync.dma_start(scv[:, j0:j0 + nb, :], scb[:, :nb, :])
    nc.compile()
    return nc


def _edge_phase(nc, pools, *, sched, feat, nh, hg, scg, ident, agg_cb):
    """hg: DRAM [P, TT*feat] bf16 (edge (p,t) -> message row).
    scg: DRAM [P, TT*2*nh] fp32 ([as|ad] per edge).
    Round-robin identity layout: partition index == dst slot within the
    block, so the aggregation matmul lhsT is the constant identity."""
    gp, mp, ap_, pp, sp = pools
    hg3 = hg.rearrange("p (t f) -> p t f", f=feat)
    sc3 = scg.rearrange("p (t s) -> p t s", s=2 * nh)
    fw = feat + nh
    pack = 4 if feat == 32 else 2      # tiles per matmul (N = pack*fw <= 512)
    fwp = pack * fw
    agg = None
    for ci, (b, t0, nt, first, last) in enumerate(sched):
        g = gp.tile([P, CH, feat], dt.bfloat16, tag="g")
        nc.sync.dma_start(g[:, :nt, :], hg3[:, t0:t0 + nt, :])
        s = ap_.tile([P, CH, 2 * nh], dt.float32, tag="s")
        nc.sync.dma_start(s[:, :nt, :], sc3[:, t0:t0 + nt, :])
        if first:
            agg = pp.tile([P, fwp], dt.float32, tag="agg")
        q = ap_.tile([P, CH, nh], dt.float32, tag="q")
        nc.vector.tensor_tensor(q[:, :nt, :], s[:, :nt, 0:nh],
                                s[:, :nt, nh:2 * nh], op=ALU.add)
        nc.vector.scalar_tensor_tensor(q[:, :nt, :], q[:, :nt, :], 0.2,
                                       q[:, :nt, :], op0=ALU.mult, op1=ALU.max)
        # w = exp(q) written (bf16) straight into msg denominator columns
        msg = mp.tile([P, CH, fw], dt.bfloat16, tag="msg")
        nc.scalar.activation(msg[:, :nt, feat:fw], q[:, :nt, :], ACTF.Exp)
        # alternate chunks between scalar-engine w-expansion (then plain
        # contiguous DVE multiply) and the fused broadcast DVE multiply,
        # balancing ACT vs DVE load
        cpf = feat // nh
        wv = _ap(msg[:, :, feat:fw], [[fw, nt], [1, nh], [0, cpf]])
        if ci % 2 == 0:
            wexp = mp.tile([P, CH, feat], dt.bfloat16, tag="wexp")
            nc.scalar.copy(wexp[:, :nt, :], wv)
            gv = _ap(g[:], [[feat, nt], [1, feat]])
            mv = _ap(msg[:], [[fw, nt], [1, feat]])
            nc.vector.tensor_tensor(
                mv, gv, _ap(wexp[:], [[feat, nt], [1, feat]]), op=ALU.mult)
        else:
            gv = _ap(g[:], [[feat, nt], [cpf, nh], [1, cpf]])
            mv = _ap(msg[:], [[fw, nt], [cpf, nh], [1, cpf]])
            nc.vector.tensor_tensor(mv, gv, wv, op=ALU.mult)
        rem = nt % pack
        if rem:
            nc.vector.memset(msg[:, nt:nt + pack - rem, :], 0.0)
        for t in range(0, nt, pack):
            nc.tensor.matmul(out=agg[:], lhsT=ident[:],
                             rhs=msg[:, t:t + pack, :],
                             start=(first and t == 0),
                             stop=(last and t + pack >= nt))
        if last:
            # fold the pack column groups into one [P, fw] SBUF tile
            aggsum = sp.tile([P, fw], dt.float32, tag="aggsum")
            nc.vector.tensor_reduce(aggsum[:],
                                    _ap(agg[:], [[1, fw], [fw, pack]]),
                                    AX, ALU.add)
            agg_cb(b, aggsum)


def build_l2(sched, TT):
    nc = _mk_nc()
    hg = nc.dram_tensor("h1g", [P, TT * P], dt.bfloat16, kind="ExternalInput").ap()
    scg = nc.dram_tensor("sc1g", [P, TT * 8], dt.float32, kind="ExternalInput").ap()
    idn = nc.dram_tensor("ident", [P, P], dt.bfloat16, kind="ExternalInput").ap()
    b1d = nc.dram_tensor("b1t", [P, P], dt.float32, kind="ExternalInput").ap()
    wc2d = nc.dram_tensor("wc2", [P, 34], dt.bfloat16, kind="ExternalInput").ap()
    h2b = nc.dram_tensor("h2b", [P, NBLK * 32], dt.bfloat16,
                         kind="ExternalOutput").ap()
    sc2 = nc.dram_tensor("sc2", [P, NBLK * 2], dt.float32,
                         kind="ExternalOutput").ap()
    with tile.TileContext(nc) as tc:
        with (
            tc.tile_pool(name="one", bufs=1) as one,
            tc.tile_pool(name="g", bufs=3) as gp,
            tc.tile_pool(name="m", bufs=3) as mp,
            tc.tile_pool(name="a", bufs=3) as ap_,
            tc.tile_pool(name="s", bufs=3) as sp,
            tc.tile_pool(name="ps", bufs=2, space="PSUM") as pp,
        ):
            ident = one.tile([P, P], dt.bfloat16)
            nc.sync.dma_start(ident[:], idn)
            b1t = one.tile([P, P], dt.float32)
            nc.sync.dma_start(b1t[:], b1d)
            wc2 = one.tile([P, 34], dt.bfloat16)
            nc.sync.dma_start(wc2[:], wc2d)
            hball = one.tile([P, NBLK, 32], dt.bfloat16)
            sball = one.tile([P, NBLK, 2], dt.float32)

            def epilogue(b, agg):
                rc = sp.tile([P, 4], dt.float32, tag="rc")
                nc.vector.reciprocal(rc[:], agg[:, P:P + 4])
                o = sp.tile([P, P], dt.float32, tag="o")
                nc.vector.tensor_tensor(o[:], agg[:, 0:P],
                                        _ap(rc[:], [[1, 4], [0, 32]]),
                                        op=ALU.mult)
                nc.vector.tensor_add(o[:], o[:], b1t[:])
                mn = sp.tile([P, P], dt.float32, tag="mn")
                nc.vector.tensor_scalar(mn[:], o[:], 0.0, None, ALU.min)
                em = sp.tile([P, P], dt.float32, tag="em")
                nc.scalar.activation(em[:], mn[:], ACTF.Exp)
                r = sp.tile([P, P], dt.float32, tag="r")
                nc.scalar.activation(r[:], o[:], ACTF.Relu)
                h1p = sp.tile([P, P], dt.bfloat16, tag="h1p")
                nc.vector.scalar_tensor_tensor(h1p[:], em[:], -1.0, r[:],
                                               op0=ALU.add, op1=ALU.add)
                trp = pp.tile([P, P], dt.bfloat16, tag="tr")
                nc.tensor.transpose(trp[:], h1p[:], ident[:])
                h1pT = sp.tile([P, P], dt.bfloat16, tag="h1pT")
                nc.vector.tensor_copy(h1pT[:], trp[:])
                h2p = pp.tile([P, 34], dt.float32, tag="h2p")
                nc.tensor.matmul(out=h2p[:], lhsT=h1pT[:], rhs=wc2[:],
                                 start=True, stop=True)
                nc.vector.tensor_copy(hball[:, b, :], h2p[:, 0:32])
                nc.vector.tensor_copy(sball[:, b, :], h2p[:, 32:34])

            _edge_phase(nc, (gp, mp, ap_, pp, sp), sched=sched, feat=P, nh=4,
                        hg=hg, scg=scg, ident=ident, agg_cb=epilogue)
            nc.sync.dma_start(h2b, hball[:].rearrange("p j f -> p (j f)"))
            nc.sync.dma_start(sc2, sball[:].rearrange("p j f -> p (j f)"))
    nc.compile()
    return nc


def build_l3(sched, TT):
    nc = _mk_nc()
    hg = nc.dram_tensor("h2g", [P, TT * 32], dt.bfloat16, kind="ExternalInput").ap()
    scg = nc.dram_tensor("sc2g", [P, TT * 2], dt.float32, kind="ExternalInput").ap()
    idn = nc.dram_tensor("ident", [P, P], dt.bfloat16, kind="ExternalInput").ap()
    b2d = nc.dram_tensor("b2t", [P, 32], dt.float32, kind="ExternalInput").ap()
    txtT = nc.dram_tensor("txtT", [768, SLOTS], dt.bfloat16, kind="ExternalInput").ap()
    wfa = nc.dram_tensor("wf1a", [768, 64], dt.bfloat16, kind="ExternalInput").ap()
    wfb = nc.dram_tensor("wf1b", [32, 64], dt.bfloat16, kind="ExternalInput").ap()
    wf2 = nc.dram_tensor("wf2", [64, 1], dt.bfloat16, kind="ExternalInput").ap()
    bf1 = nc.dram_tensor("bf1c", [64, 1], dt.float32, kind="ExternalInput").ap()
    bf2v = nc.dram_tensor("bf2v", [1, 1], dt.float32, kind="ExternalInput").ap()
    outd = nc.dram_tensor("out", [1, SLOTS], dt.float32, kind="ExternalOutput").ap()
    with tile.TileContext(nc) as tc:
        with (
            tc.tile_pool(name="one", bufs=1) as one,
            tc.tile_pool(name="g", bufs=3) as gp,
            tc.tile_pool(name="m", bufs=3) as mp,
            tc.tile_pool(name="a", bufs=3) as ap_,
            tc.tile_pool(name="s", bufs=3) as sp,
            tc.tile_pool(name="ps", bufs=2, space="PSUM") as pp,
        ):
            ident = one.tile([P, P], dt.bfloat16)
            nc.sync.dma_start(ident[:], idn)
            b2t = one.tile([P, 32], dt.float32)
            nc.sync.dma_start(b2t[:], b2d)
            wf1a = []
            for k in range(6):
                w = one.tile([P, 64], dt.bfloat16, tag=f"wfa{k}")
                nc.sync.dma_start(w[:], wfa[k * P:(k + 1) * P, :])
                wf1a.append(w)
            wf1b = one.tile([32, 64], dt.bfloat16)
            nc.sync.dma_start(wf1b[:], wfb)
            wf2c = one.tile([64, 1], dt.bfloat16)
            nc.sync.dma_start(wf2c[:], wf2)
            bf1c = one.tile([64, 1], dt.float32)
            nc.sync.dma_start(bf1c[:], bf1)
            bf2t = one.tile([1, 1], dt.float32)
            nc.sync.dma_start(bf2t[:], bf2v)
            osb = one.tile([1, SLOTS], dt.float32)
            txall = one.tile([P, 6, SLOTS], dt.bfloat16)
            for k in range(6):
                nc.sync.dma_start(txall[:, k, :], txtT[k * P:(k + 1) * P, :])

            def epilogue(b, agg):
                rc = sp.tile([P, 1], dt.float32, tag="rc")
                nc.vector.reciprocal(rc[:], agg[:, 32:33])
                gf = sp.tile([P, 32], dt.float32, tag="gf")
                nc.vector.tensor_tensor(gf[:], agg[:, 0:32],
                                        _ap(rc[:], [[0, 32]]),
                                        op=ALU.mult)
                nc.vector.tensor_add(gf[:], gf[:], b2t[:])
                gb = sp.tile([P, 32], dt.bfloat16, tag="gb")
                nc.vector.tensor_copy(gb[:], gf[:])
                trp = pp.tile([32, P], dt.bfloat16, tag="tr")
                nc.tensor.transpose(trp[:], gb[:], ident[:])
                gT = sp.tile([32, P], dt.bfloat16, tag="gT")
                nc.vector.tensor_copy(gT[:], trp[:])
                zT = pp.tile([64, P], dt.float32, tag="zT")
                for k in range(6):
                    nc.tensor.matmul(out=zT[:], lhsT=wf1a[k][:],
                                     rhs=txall[:, k, b * P:(b + 1) * P],
                                     start=(k == 0), stop=False)
                nc.tensor.matmul(out=zT[:], lhsT=wf1b[:], rhs=gT[:],
                                 start=False, stop=True)
                zs = sp.tile([64, P], dt.bfloat16, tag="zs")
                nc.scalar.activation(zs[:], zT[:], ACTF.Relu, bias=bf1c[:])
                op = pp.tile([1, P], dt.float32, tag="op")
                nc.tensor.matmul(out=op[:], lhsT=wf2c[:], rhs=zs[:],
                                 start=True, stop=True)
                nc.vector.tensor_tensor(osb[0:1, b * P:(b + 1) * P], op[:],
                                        _ap(bf2t[:], [[0, P]]), op=ALU.add)

            _edge_phase(nc, (gp, mp, ap_, pp, sp), sched=sched, feat=32, nh=1,
                        hg=hg, scg=scg, ident=ident, agg_cb=epilogue)
            nc.sync.dma_start(outd, osb[:])
    nc.compile()
    return nc


# ------------------------------------------------------------------ runner
def kernel(txt, x, W1, a_src1, a_dst1, b1, W2, a_src2, a_dst2, b2,
           Wf1, bf1, Wf2, bf2, edge_index, _trace=False, _times=None):
    txt = np.asarray(txt, np.float32)
    x = np.asarray(x, np.float32)
    Wc1, Wc2 = _fold_weights(
        np.asarray(W1, np.float32), np.asarray(a_src1, np.float32),
        np.asarray(a_dst1, np.float32), np.asarray(W2, np.float32),
        np.asarray(a_src2, np.float32), np.asarray(a_dst2, np.float32))
    pos_src, pos_pad, sched, TT, tile2blk, orders = _edge_prep(
        np.asarray(edge_index))

    ident_t = np.eye(P, dtype=np.float32).astype(BF16)
    b1t = np.tile(np.asarray(b1, np.float32), (P, 1)).astype(np.float32)
    b2t = np.tile(np.asarray(b2, np.float32), (P, 1)).astype(np.float32)
    Wf1 = np.asarray(Wf1, np.float32)
    wf1a = np.ascontiguousarray(Wf1[:768]).astype(BF16)
    wf1b = np.ascontiguousarray(Wf1[768:]).astype(BF16)
    wf2 = np.asarray(Wf2, np.float32).reshape(64, 1).astype(BF16)
    bf1c = np.asarray(bf1, np.float32).reshape(64, 1)
    bf2v = np.asarray(bf2, np.float32).reshape(1, 1)

    def run(nc, ins):
        res = bass_utils.run_bass_kernel_spmd(
            nc, ins, core_ids=list(range(NCORE)), trace=_trace)
        if _times is not None:
            _times.append(res.exec_time_ns)
        return res

    # dst node (global orig id) per (p, t) position, for ad gathers
    dstg = []
    for p in range(NCORE):
        slot = tile2blk[None, :] * P + np.arange(P, dtype=np.int64)[:, None]
        dstg.append(p * NPC + orders[p][np.minimum(slot, NPC - 1)])

    # ---------------- launch 1
    nc1 = build_l1()
    xb = x.astype(BF16)
    in1 = []
    for p in range(NCORE):
        xs = np.zeros((512, SLOTS), BF16)
        xs[:, :NPC] = xb[p * NPC:(p + 1) * NPC][orders[p]].T
        in1.append({"xT": xs, "wc1": Wc1})
    r1 = run(nc1, in1)

    def unmaj(a, f):
        return a.reshape(P, NBLK, f).transpose(1, 0, 2).reshape(SLOTS, f)[:NPC]

    h1_full = np.empty((N, P), BF16)
    sc1_full = np.empty((N, 8), np.float32)
    for p in range(NCORE):
        h1_full[p * NPC + orders[p]] = unmaj(r1.results[p]["h1b"], P)
        sc1_full[p * NPC + orders[p]] = unmaj(r1.results[p]["sc"], 8)

    # ---------------- halo gather 1 + launch 2
    nc2 = build_l2(sched, TT)
    in2 = []
    for p in range(NCORE):
        h1g = h1_full[pos_src[p]].reshape(P, TT * P)
        as1 = sc1_full[pos_src[p], 0:4].copy()
        as1[pos_pad[p]] = -1e5            # pad edges -> zero weight
        sc1g = np.concatenate(
            [as1, sc1_full[dstg[p], 4:8]],
            axis=2).astype(np.float32).reshape(P, TT * 8)
        in2.append({"h1g": h1g, "sc1g": sc1g,
                    "ident": ident_t, "b1t": b1t, "wc2": Wc2})
    r2 = run(nc2, in2)
    h2_full = np.empty((N, 32), BF16)
    sc2_full = np.empty((N, 2), np.float32)
    for p in range(NCORE):
        h2_full[p * NPC + orders[p]] = unmaj(r2.results[p]["h2b"], 32)
        sc2_full[p * NPC + orders[p]] = unmaj(r2.results[p]["sc2"], 2)

    # ---------------- halo gather 2 + launch 3
    nc3 = build_l3(sched, TT)
    txtb = txt.astype(BF16)
    in3 = []
    for p in range(NCORE):
        h2g = h2_full[pos_src[p]].reshape(P, TT * 32)
        as2 = sc2_full[pos_src[p], 0:1].copy()
        as2[pos_pad[p]] = -1e5
        sc2g = np.concatenate(
            [as2, sc2_full[dstg[p], 1:2]],
            axis=2).astype(np.float32).reshape(P, TT * 2)
        ts = np.zeros((768, SLOTS), BF16)
        ts[:, :NPC] = txtb[p * NPC:(p + 1) * NPC][orders[p]].T
        in3.append({"h2g": h2g, "sc2g": sc2g,
                    "ident": ident_t, "b2t": b2t, "txtT": ts,
                    "wf1a": wf1a, "wf1b": wf1b, "wf2": wf2, "bf1c": bf1c,
                    "bf2v": bf2v})
    r3 = run(nc3, in3)
    out = np.empty(N, np.float32)
    for p in range(NCORE):
        out[p * NPC + orders[p]] = r3.results[p]["out"][0, :NPC]
    return out.astype(np.float32)


# revision 34
# speedup vs baseline: 1.0303x; 1.0303x over previous
"""GAT fusion Trainium2 kernel — nn_GAT_Fusion_2147483648587.

Full inputs in, full output out. Node/dst-sharded across 8 NeuronCores
(graph parallel); per-edge source features are host-gathered between
three device launches (halo exchange); all FLOPs run on device.

  launch 1: per-core dense  h1 = x@W1 (+folded attention score vectors)
  host    : gather h1[src], as1[src], ad1[dst] per edge (pure indexing)
  launch 2: L1 edge phase — exp(lrelu(scores)) weights, one-hot-mask
            matmul aggregation in PSUM, softmax-normalize, ELU,
            h2 = h1'@W2 (+folded L2 score vectors)
  host    : gather h2[src], as2[src], ad2[dst] per edge
  launch 3: L2 edge phase + fusion MLP -> out[N]

Segment softmax computed without max-subtraction (exactly equivalent —
the per-dst scale cancels between numerator and denominator; scores are
O(10) so fp32 exp cannot overflow).
"""
import sys

sys.path.insert(0, "/opt/trn_rl_repo")

import numpy as np
import ml_dtypes

import concourse.bass as bass
from concourse import bacc
import concourse.tile as tile
from concourse import mybir
from concourse import bass_utils

BF16 = ml_dtypes.bfloat16
P = 128
NCORE = 8
N = 50000
NPC = N // NCORE            # 6250 dst nodes per core
NBLK = (NPC + P - 1) // P   # 49 dst blocks per core
SLOTS = NBLK * P            # 6272
CH = 32                     # tiles per chunk (4096 edges)
dt = mybir.dt
AX = mybir.AxisListType.X
ALU = mybir.AluOpType
ACTF = mybir.ActivationFunctionType


def _ap(tile_ap, dims):
    return bass.AP(tensor=tile_ap.tensor, offset=tile_ap.offset,
                   ap=[tile_ap.ap[0]] + dims)


# ----------------------------------------------------------------- host prep
def _fold_weights(W1, a_src1, a_dst1, W2, a_src2, a_dst2):
    Ws1 = np.stack([W1[:, h * 32:(h + 1) * 32] @ a_src1[h] for h in range(4)], 1)
    Wd1 = np.stack([W1[:, h * 32:(h + 1) * 32] @ a_dst1[h] for h in range(4)], 1)
    Wc1 = np.concatenate([W1, Ws1, Wd1], axis=1)          # [512, 136]
    Ws2 = (W2 @ a_src2[0])[:, None]
    Wd2 = (W2 @ a_dst2[0])[:, None]
    Wc2 = np.concatenate([W2, Ws2, Wd2], axis=1)          # [128, 34]
    return Wc1.astype(BF16), Wc2.astype(BF16)


def _edge_prep(edge_index):
    """Round-robin identity layout: per core, dsts are permuted by
    descending degree into slots; the edge at (partition c, tile r of
    block b) targets dst slot b*128+c, so the aggregation mask is the
    constant identity. Tile counts per block shared by all cores."""
    loops = np.arange(N, dtype=np.int64)
    src = np.concatenate([edge_index[0], loops]).astype(np.int64)
    dst = np.concatenate([edge_index[1], loops]).astype(np.int64)
    deg = np.bincount(dst, minlength=N)
    owner = dst // NPC

    orders = []
    Tb = np.ones(NBLK, np.int64)
    for p in range(NCORE):
        d = deg[p * NPC:(p + 1) * NPC]
        order = np.argsort(-d, kind="stable")        # slot -> local node
        orders.append(order)
        ds = np.concatenate([d[order], np.zeros(SLOTS - NPC, np.int64)])
        Tb = np.maximum(Tb, ds.reshape(NBLK, P).max(axis=1))

    tstart = np.zeros(NBLK + 1, np.int64)
    tstart[1:] = np.cumsum(Tb)
    TT = int(tstart[-1])

    sched = []   # (blk, tile_start, ntiles, first_of_blk, last_of_blk)
    for b in range(NBLK):
        t0, rem, off = int(tstart[b]), int(Tb[b]), 0
        while rem > 0:
            nt = min(CH, rem)
            sched.append((b, t0 + off, nt, off == 0, rem - nt == 0))
            off += nt
            rem -= nt

    tile2blk = np.zeros(TT, np.int64)
    for b in range(NBLK):
        tile2blk[tstart[b]:tstart[b + 1]] = b

    pos_src, pos_pad = [], []
    for p in range(NCORE):
        rank = np.empty(NPC, np.int64)
        rank[orders[p]] = np.arange(NPC)
        sel = owner == p
        s_p = src[sel]
        slot = rank[dst[sel] - p * NPC]
        o = np.argsort(slot, kind="stable")
        s_p, slot = s_p[o], slot[o]
        run0 = np.concatenate(([0], np.cumsum(np.bincount(slot,
                                                          minlength=SLOTS))))
        r = np.arange(len(slot)) - run0[slot]        # round within slot
        tcol = tstart[slot >> 7] + r
        crow = slot & 127
        srcp = np.zeros((P, TT), np.int64)
        pad = np.ones((P, TT), bool)
        srcp[crow, tcol] = s_p
        pad[crow, tcol] = False
        pos_src.append(srcp)
        pos_pad.append(pad)
    return pos_src, pos_pad, sched, TT, tile2blk, orders


# ------------------------------------------------------------- bass builders
def _mk_nc():
    return bacc.Bacc("TRN2", target_bir_lowering=False, debug=False,
                     num_devices=NCORE)


def build_l1():
    nc = _mk_nc()
    xT = nc.dram_tensor("xT", [512, SLOTS], dt.bfloat16, kind="ExternalInput").ap()
    wc1 = nc.dram_tensor("wc1", [512, 136], dt.bfloat16, kind="ExternalInput").ap()
    h1b = nc.dram_tensor("h1b", [P, NBLK * P], dt.bfloat16,
                         kind="ExternalOutput").ap()
    sc = nc.dram_tensor("sc", [P, NBLK * 8], dt.float32,
                        kind="ExternalOutput").ap()
    with tile.TileContext(nc) as tc:
        with (
            tc.tile_pool(name="w", bufs=1) as wp,
            tc.tile_pool(name="x", bufs=4) as xp,
            tc.tile_pool(name="s", bufs=3) as sp,
            tc.tile_pool(name="ps", bufs=3, space="PSUM") as pp,
        ):
            wk = []
            for k in range(4):
                w = wp.tile([P, 136], dt.bfloat16, tag=f"w{k}")
                nc.sync.dma_start(w[:], wc1[k * P:(k + 1) * P, :])
                wk.append(w)
            GRP = 8                       # blocks per x-DMA batch
            for j0 in range(0, NBLK, GRP):
                nb = min(GRP, NBLK - j0)
                xt = xp.tile([P, 4, GRP * P], dt.bfloat16, tag="xt")
                for k in range(4):
                    nc.sync.dma_start(
                        xt[:, k, :nb * P],
                        xT[k * P:(k + 1) * P, j0 * P:(j0 + nb) * P])
                hb = sp.tile([P, GRP, P], dt.bfloat16, tag="hb")
                scb = sp.tile([P, GRP, 8], dt.float32, tag="scb")
                for j in range(j0, j0 + nb):
                    o = j - j0
                    ps = pp.tile([P, 136], dt.float32, tag="p1")
                    for k in range(4):
                        nc.tensor.matmul(out=ps[:], lhsT=xt[:, k, o * P:(o + 1) * P],
                                         rhs=wk[k][:],
                                         start=(k == 0), stop=(k == 3))
                    nc.vector.tensor_copy(hb[:, o, :], ps[:, 0:P])
                    nc.vector.tensor_copy(scb[:, o, :], ps[:, P:136])
                h1v = h1b.rearrange("p (j f) -> p j f", f=P)
                scv = sc.rearrange("p (j f) -> p j f", f=8)
                nc.sync.dma_start(h1v[:, j0:j0 + nb, :], hb[:, :nb, :])
                nc.sync.dma_start(scv[:, j0:j0 + nb, :], scb[:, :nb, :])
    nc.compile()
    return nc


def _edge_phase(nc, pools, *, sched, feat, nh, hg, scg, ident, agg_cb):
    """hg: DRAM [P, TT*feat] bf16 (edge (p,t) -> message row).
    scg: DRAM [P, TT*2*nh] fp32 ([as|ad] per edge).
    Round-robin identity layout: partition index == dst slot within the
    block, so the aggregation matmul lhsT is the constant identity."""
    gp, mp, ap_, pp, sp = pools
    hg3 = hg.rearrange("p (t f) -> p t f", f=feat)
    sc3 = scg.rearrange("p (t s) -> p t s", s=2 * nh)
    fw = feat + nh
    agg = None
    for ci, (b, t0, nt, first, last) in enumerate(sched):
        g = gp.tile([P, CH, feat], dt.bfloat16, tag="g")
        nc.sync.dma_start(g[:, :nt, :], hg3[:, t0:t0 + nt, :])
        s = ap_.tile([P, CH, 2 * nh], dt.float32, tag="s")
        nc.sync.dma_start(s[:, :nt, :], sc3[:, t0:t0 + nt, :])
        if first:
            agg = pp.tile([P, fw], dt.float32, tag="agg")
        q = ap_.tile([P, CH, nh], dt.float32, tag="q")
        nc.vector.tensor_tensor(q[:, :nt, :], s[:, :nt, 0:nh],
                                s[:, :nt, nh:2 * nh], op=ALU.add)
        nc.vector.scalar_tensor_tensor(q[:, :nt, :], q[:, :nt, :], 0.2,
                                       q[:, :nt, :], op0=ALU.mult, op1=ALU.max)
        # w = exp(q) written (bf16) straight into msg denominator columns
        msg = mp.tile([P, CH, fw], dt.bfloat16, tag="msg")
        nc.scalar.activation(msg[:, :nt, feat:fw], q[:, :nt, :], ACTF.Exp)
        # alternate chunks between scalar-engine w-expansion (then plain
        # contiguous DVE multiply) and the fused broadcast DVE multiply,
        # balancing ACT vs DVE load
        cpf = feat // nh
        wv = _ap(msg[:, :, feat:fw], [[fw, nt], [1, nh], [0, cpf]])
        if ci % 2 == 0:
            wexp = mp.tile([P, CH, feat], dt.bfloat16, tag="wexp")
            nc.scalar.copy(wexp[:, :nt, :], wv)
            gv = _ap(g[:], [[feat, nt], [1, feat]])
            mv = _ap(msg[:], [[fw, nt], [1, feat]])
            nc.vector.tensor_tensor(
                mv, gv, _ap(wexp[:], [[feat, nt], [1, feat]]), op=ALU.mult)
        else:
            gv = _ap(g[:], [[feat, nt], [cpf, nh], [1, cpf]])
            mv = _ap(msg[:], [[fw, nt], [cpf, nh], [1, cpf]])
            nc.vector.tensor_tensor(mv, gv, wv, op=ALU.mult)
        for t in range(nt):
            nc.tensor.matmul(out=agg[:], lhsT=ident[:], rhs=msg[:, t, :],
                             start=(first and t == 0),
                             stop=(last and t == nt - 1))
        if last:
            agg_cb(b, agg)


def build_l2(sched, TT):
    nc = _mk_nc()
    hg = nc.dram_tensor("h1g", [P, TT * P], dt.bfloat16, kind="ExternalInput").ap()
    scg = nc.dram_tensor("sc1g", [P, TT * 8], dt.float32, kind="ExternalInput").ap()
    idn = nc.dram_tensor("ident", [P, P], dt.bfloat16, kind="ExternalInput").ap()
    b1d = nc.dram_tensor("b1t", [P, P], dt.float32, kind="ExternalInput").ap()
    wc2d = nc.dram_tensor("wc2", [P, 34], dt.bfloat16, kind="ExternalInput").ap()
    h2b = nc.dram_tensor("h2b", [P, NBLK * 32], dt.bfloat16,
                         kind="ExternalOutput").ap()
    sc2 = nc.dram_tensor("sc2", [P, NBLK * 2], dt.float32,
                         kind="ExternalOutput").ap()
    with tile.TileContext(nc) as tc:
        with (
            tc.tile_pool(name="one", bufs=1) as one,
            tc.tile_pool(name="g", bufs=5) as gp,
            tc.tile_pool(name="m", bufs=4) as mp,
            tc.tile_pool(name="a", bufs=5) as ap_,
            tc.tile_pool(name="s", bufs=3) as sp,
            tc.tile_pool(name="ps", bufs=2, space="PSUM") as pp,
        ):
            ident = one.tile([P, P], dt.bfloat16)
            nc.sync.dma_start(ident[:], idn)
            b1t = one.tile([P, P], dt.float32)
            nc.sync.dma_start(b1t[:], b1d)
            wc2 = one.tile([P, 34], dt.bfloat16)
            nc.sync.dma_start(wc2[:], wc2d)
            hball = one.tile([P, NBLK, 32], dt.bfloat16)
            sball = one.tile([P, NBLK, 2], dt.float32)

            def epilogue(b, agg):
                rc = sp.tile([P, 4], dt.float32, tag="rc")
                nc.vector.reciprocal(rc[:], agg[:, P:P + 4])
                o = sp.tile([P, P], dt.float32, tag="o")
                nc.vector.tensor_tensor(o[:], agg[:, 0:P],
                                        _ap(rc[:], [[1, 4], [0, 32]]),
                                        op=ALU.mult)
                nc.vector.tensor_add(o[:], o[:], b1t[:])
                mn = sp.tile([P, P], dt.float32, tag="mn")
                nc.vector.tensor_scalar(mn[:], o[:], 0.0, None, ALU.min)
                em = sp.tile([P, P], dt.float32, tag="em")
                nc.scalar.activation(em[:], mn[:], ACTF.Exp)
                r = sp.tile([P, P], dt.float32, tag="r")
                nc.scalar.activation(r[:], o[:], ACTF.Relu)
                h1p = sp.tile([P, P], dt.bfloat16, tag="h1p")
                nc.vector.scalar_tensor_tensor(h1p[:], em[:], -1.0, r[:],
                                               op0=ALU.add, op1=ALU.add)
                trp = pp.tile([P, P], dt.bfloat16, tag="tr")
                nc.tensor.transpose(trp[:], h1p[:], ident[:])
                h1pT = sp.tile([P, P], dt.bfloat16, tag="h1pT")
                nc.vector.tensor_copy(h1pT[:], trp[:])
                h2p = pp.tile([P, 34], dt.float32, tag="h2p")
                nc.tensor.matmul(out=h2p[:], lhsT=h1pT[:], rhs=wc2[:],
                                 start=True, stop=True)
                nc.vector.tensor_copy(hball[:, b, :], h2p[:, 0:32])
                nc.vector.tensor_copy(sball[:, b, :], h2p[:, 32:34])

            _edge_phase(nc, (gp, mp, ap_, pp, sp), sched=sched, feat=P, nh=4,
                        hg=hg, scg=scg, ident=ident, agg_cb=epilogue)
            nc.sync.dma_start(h2b, hball[:].rearrange("p j f -> p (j f)"))
            nc.sync.dma_start(sc2, sball[:].rearrange("p j f -> p (j f)"))
    nc.compile()
    return nc


def build_l3(sched, TT):
    nc = _mk_nc()
    hg = nc.dram_tensor("h2g", [P, TT * 32], dt.bfloat16, kind="ExternalInput").ap()
    scg = nc.dram_tensor("sc2g", [P, TT * 2], dt.float32, kind="ExternalInput").ap()
    idn = nc.dram_tensor("ident", [P, P], dt.bfloat16, kind="ExternalInput").ap()
    b2d = nc.dram_tensor("b2t", [P, 32], dt.float32, kind="ExternalInput").ap()
    txtT = nc.dram_tensor("txtT", [768, SLOTS], dt.bfloat16, kind="ExternalInput").ap()
    wfa = nc.dram_tensor("wf1a", [768, 64], dt.bfloat16, kind="ExternalInput").ap()
    wfb = nc.dram_tensor("wf1b", [32, 64], dt.bfloat16, kind="ExternalInput").ap()
    wf2 = nc.dram_tensor("wf2", [64, 1], dt.bfloat16, kind="ExternalInput").ap()
    bf1 = nc.dram_tensor("bf1c", [64, 1], dt.float32, kind="ExternalInput").ap()
    bf2v = nc.dram_tensor("bf2v", [1, 1], dt.float32, kind="ExternalInput").ap()
    outd = nc.dram_tensor("out", [1, SLOTS], dt.float32, kind="ExternalOutput").ap()
    with tile.TileContext(nc) as tc:
        with (
            tc.tile_pool(name="one", bufs=1) as one,
            tc.tile_pool(name="g", bufs=5) as gp,
            tc.tile_pool(name="m", bufs=4) as mp,
            tc.tile_pool(name="a", bufs=5) as ap_,
            tc.tile_pool(name="s", bufs=3) as sp,
            tc.tile_pool(name="ps", bufs=2, space="PSUM") as pp,
        ):
            ident = one.tile([P, P], dt.bfloat16)
            nc.sync.dma_start(ident[:], idn)
            b2t = one.tile([P, 32], dt.float32)
            nc.sync.dma_start(b2t[:], b2d)
            wf1a = []
            for k in range(6):
                w = one.tile([P, 64], dt.bfloat16, tag=f"wfa{k}")
                nc.sync.dma_start(w[:], wfa[k * P:(k + 1) * P, :])
                wf1a.append(w)
            wf1b = one.tile([32, 64], dt.bfloat16)
            nc.sync.dma_start(wf1b[:], wfb)
            wf2c = one.tile([64, 1], dt.bfloat16)
            nc.sync.dma_start(wf2c[:], wf2)
            bf1c = one.tile([64, 1], dt.float32)
            nc.sync.dma_start(bf1c[:], bf1)
            bf2t = one.tile([1, 1], dt.float32)
            nc.sync.dma_start(bf2t[:], bf2v)
            osb = one.tile([1, SLOTS], dt.float32)
            txall = one.tile([P, 6, SLOTS], dt.bfloat16)
            for k in range(6):
                nc.sync.dma_start(txall[:, k, :], txtT[k * P:(k + 1) * P, :])

            def epilogue(b, agg):
                rc = sp.tile([P, 1], dt.float32, tag="rc")
                nc.vector.reciprocal(rc[:], agg[:, 32:33])
                gf = sp.tile([P, 32], dt.float32, tag="gf")
                nc.vector.tensor_tensor(gf[:], agg[:, 0:32],
                                        _ap(rc[:], [[0, 32]]),
                                        op=ALU.mult)
                nc.vector.tensor_add(gf[:], gf[:], b2t[:])
                gb = sp.tile([P, 32], dt.bfloat16, tag="gb")
                nc.vector.tensor_copy(gb[:], gf[:])
                trp = pp.tile([32, P], dt.bfloat16, tag="tr")
                nc.tensor.transpose(trp[:], gb[:], ident[:])
                gT = sp.tile([32, P], dt.bfloat16, tag="gT")
                nc.vector.tensor_copy(gT[:], trp[:])
                zT = pp.tile([64, P], dt.float32, tag="zT")
                for k in range(6):
                    nc.tensor.matmul(out=zT[:], lhsT=wf1a[k][:],
                                     rhs=txall[:, k, b * P:(b + 1) * P],
                                     start=(k == 0), stop=False)
                nc.tensor.matmul(out=zT[:], lhsT=wf1b[:], rhs=gT[:],
                                 start=False, stop=True)
                zs = sp.tile([64, P], dt.bfloat16, tag="zs")
                nc.scalar.activation(zs[:], zT[:], ACTF.Relu, bias=bf1c[:])
                op = pp.tile([1, P], dt.float32, tag="op")
                nc.tensor.matmul(out=op[:], lhsT=wf2c[:], rhs=zs[:],
                                 start=True, stop=True)
                nc.vector.tensor_tensor(osb[0:1, b * P:(b + 1) * P], op[:],
                                        _ap(bf2t[:], [[0, P]]), op=ALU.add)

            _edge_phase(nc, (gp, mp, ap_, pp, sp), sched=sched, feat=32, nh=1,
                        hg=hg, scg=scg, ident=ident, agg_cb=epilogue)
            nc.sync.dma_start(outd, osb[:])
    nc.compile()
    return nc


# ------------------------------------------------------------------ runner
def kernel(txt, x, W1, a_src1, a_dst1, b1, W2, a_src2, a_dst2, b2,
           Wf1, bf1, Wf2, bf2, edge_index, _trace=False, _times=None):
    txt = np.asarray(txt, np.float32)
    x = np.asarray(x, np.float32)
    Wc1, Wc2 = _fold_weights(
        np.asarray(W1, np.float32), np.asarray(a_src1, np.float32),
        np.asarray(a_dst1, np.float32), np.asarray(W2, np.float32),
        np.asarray(a_src2, np.float32), np.asarray(a_dst2, np.float32))
    pos_src, pos_pad, sched, TT, tile2blk, orders = _edge_prep(
        np.asarray(edge_index))

    ident_t = np.eye(P, dtype=np.float32).astype(BF16)
    b1t = np.tile(np.asarray(b1, np.float32), (P, 1)).astype(np.float32)
    b2t = np.tile(np.asarray(b2, np.float32), (P, 1)).astype(np.float32)
    Wf1 = np.asarray(Wf1, np.float32)
    wf1a = np.ascontiguousarray(Wf1[:768]).astype(BF16)
    wf1b = np.ascontiguousarray(Wf1[768:]).astype(BF16)
    wf2 = np.asarray(Wf2, np.float32).reshape(64, 1).astype(BF16)
    bf1c = np.asarray(bf1, np.float32).reshape(64, 1)
    bf2v = np.asarray(bf2, np.float32).reshape(1, 1)

    def run(nc, ins):
        res = bass_utils.run_bass_kernel_spmd(
            nc, ins, core_ids=list(range(NCORE)), trace=_trace)
        if _times is not None:
            _times.append(res.exec_time_ns)
        return res

    # dst node (global orig id) per (p, t) position, for ad gathers
    dstg = []
    for p in range(NCORE):
        slot = tile2blk[None, :] * P + np.arange(P, dtype=np.int64)[:, None]
        dstg.append(p * NPC + orders[p][np.minimum(slot, NPC - 1)])

    # ---------------- launch 1
    nc1 = build_l1()
    xb = x.astype(BF16)
    in1 = []
    for p in range(NCORE):
        xs = np.zeros((512, SLOTS), BF16)
        xs[:, :NPC] = xb[p * NPC:(p + 1) * NPC][orders[p]].T
        in1.append({"xT": xs, "wc1": Wc1})
    r1 = run(nc1, in1)

    def unmaj(a, f):
        return a.reshape(P, NBLK, f).transpose(1, 0, 2).reshape(SLOTS, f)[:NPC]

    h1_full = np.empty((N, P), BF16)
    sc1_full = np.empty((N, 8), np.float32)
    for p in range(NCORE):
        h1_full[p * NPC + orders[p]] = unmaj(r1.results[p]["h1b"], P)
        sc1_full[p * NPC + orders[p]] = unmaj(r1.results[p]["sc"], 8)

    # ---------------- halo gather 1 + launch 2
    nc2 = build_l2(sched, TT)
    in2 = []
    for p in range(NCORE):
        h1g = h1_full[pos_src[p]].reshape(P, TT * P)
        as1 = sc1_full[pos_src[p], 0:4].copy()
        as1[pos_pad[p]] = -1e5            # pad edges -> zero weight
        sc1g = np.concatenate(
            [as1, sc1_full[dstg[p], 4:8]],
            axis=2).astype(np.float32).reshape(P, TT * 8)
        in2.append({"h1g": h1g, "sc1g": sc1g,
                    "ident": ident_t, "b1t": b1t, "wc2": Wc2})
    r2 = run(nc2, in2)
    h2_full = np.empty((N, 32), BF16)
    sc2_full = np.empty((N, 2), np.float32)
    for p in range(NCORE):
        h2_full[p * NPC + orders[p]] = unmaj(r2.results[p]["h2b"], 32)
        sc2_full[p * NPC + orders[p]] = unmaj(r2.results[p]["sc2"], 2)

    # ---------------- halo gather 2 + launch 3
    nc3 = build_l3(sched, TT)
    txtb = txt.astype(BF16)
    in3 = []
    for p in range(NCORE):
        h2g = h2_full[pos_src[p]].reshape(P, TT * 32)
        as2 = sc2_full[pos_src[p], 0:1].copy()
        as2[pos_pad[p]] = -1e5
        sc2g = np.concatenate(
            [as2, sc2_full[dstg[p], 1:2]],
            axis=2).astype(np.float32).reshape(P, TT * 2)
        ts = np.zeros((768, SLOTS), BF16)
        ts[:, :NPC] = txtb[p * NPC:(p + 1) * NPC][orders[p]].T
        in3.append({"h2g": h2g, "sc2g": sc2g,
                    "ident": ident_t, "b2t": b2t, "txtT": ts,
                    "wf1a": wf1a, "wf1b": wf1b, "wf2": wf2, "bf1c": bf1c,
                    "bf2v": bf2v})
    r3 = run(nc3, in3)
    out = np.empty(N, np.float32)
    for p in range(NCORE):
        out[p * NPC + orders[p]] = r3.results[p]["out"][0, :NPC]
    return out.astype(np.float32)
